# revision 43
# baseline (speedup 1.0000x reference)
"""Trainium2 Bass kernel for nn_AttentionBlock (blockwise local attention).

Per batch and head this is sliding-window causal attention with window 13
(query p attends to keys p-12..p), plus a relative-position logit term,
tanh soft-capping at 50, and key-validity masking.

Sharding: 8 cores = 4 batches x 2 T-halves. Each core computes all 8 heads
for 4080 queries of one batch half (12-row K/V halo) from host-pretransposed
x^T and the projection weights (q-scale folded into wq on host).

Per-core pipeline (bf16 matmuls, fp32 accumulation), chunked 116 queries at
a time so each chunk's 128-wide key band fits the PSUM partition dim:
  - Q^T,K^T = W^T @ x^T per head (PE); V in natural [key, head] layout (PE);
    rel logits r[q,F]=q_q.se_F (PE) interleaved per head into one psum bank
  - rel logits are scattered (DVE, f32->bf16) into persistent -1e9-poisoned
    SBUF row tiles and round-tripped through two independent DRAM skew
    planes (4 heads each, row pitch 514 written / 513 read back): the
    pitch-shifted re-read returns band[q,n,j] = rel value when
    0 <= q-j < 13 and -1e9 otherwise, i.e. the rel-position term plus the
    causal/window mask in one bf16 tile; separate DRAM tensors per plane
    keep each read gated only on its own plane's write
  - per subchunk: V matmuls first (band-latency cover), then per head
    S^T[j,q] = k_j.q_q into packed PSUM (4 heads/bank) with the upcast
    band tile transposed by the PE into the same accumulation group
    (start/stop pairs stay adjacent: start resets the bank's group)
  - tanh, exp (ACT, fused across heads) -> P^T in SBUF, bf16
  - P^T @ [V | 1] per head accumulates numerator and denominator (PE,
    deferred one subchunk so the in-order PE queue stays fed); reciprocal +
    broadcast multiply (DVE, bf16 out) -> two 1KB-row strided stores

Engine-queue discipline (the real bottlenecks were in-order queues and
DMA-engine assignment, not bandwidth): qt copies on DVE, kt copies on ACT,
x/weight loads on the SP HWDGE queue, and the whole band roundtrip plus
the output stores issued via gpsimd SWDGE -- SWDGE traffic spreads across
all 16 DMA engines (HWDGE pinned rbuf/out traffic to 4) and keeps DMA
issue off the ACT/SP queues. The bf16->f32 band upcast runs late (in the
attention loop) so in-order DVE progress never couples PV tails to
band-read DMA latency. PSUM pools are split (projections 2 / S+rel 2 /
V+PV 4 banks) so bank rotation never makes a projection wait on attention
consumers.
"""
import sys
import numpy as np

sys.path.insert(0, "/opt/trn_rl_repo")
import ml_dtypes  # noqa: E402

BF = ml_dtypes.bfloat16

B, T, D = 4, 8160, 1024
NH, HD = 8, 128
HALO = 12
CAP = 50.0
QSC = 1.0 / np.sqrt(HD)

TLOC = T // 2         # 4080 queries per core
KLOC = TLOC + HALO    # 4092
OUTER = 464           # queries per outer chunk (4 subchunks of 116)
MW = 116              # queries per attention subchunk (band = 116+12 = 128)
NF = 13               # relative-position offsets
ROWP = 4 * 128 + 2    # 514: skew-plane row pitch (4 heads per plane); the
                      # extra poison+gap cols keep the DRAM side strided so
                      # the DGE sprays all 16 engines
RDP = ROWP - 1        # 513: skewed read row pitch
ODP = D + 8           # 1032: padded out row pitch (strided store, 16 engines)
NSLOT = 8             # ring slots per plane (one per in-flight subchunk)
SLOTSZ = MW * ROWP    # bf16 elements per ring slot
NEG = -1.0e9


def _chunks_of(total, size):
    out = []
    o = 0
    while o < total:
        out.append((o, min(size, total - o)))
        o += size
    return out


def _build_program():
    import concourse.bass as bass
    import concourse.tile as tile
    from concourse import mybir
    from contextlib import ExitStack

    f32 = mybir.dt.float32
    bf16 = mybir.dt.bfloat16
    AF = mybir.ActivationFunctionType

    nc = bass.Bass(target_bir_lowering=False, debug=False)

    xT = nc.dram_tensor("xT", [D, KLOC], bf16, kind="ExternalInput")
    out = nc.dram_tensor("out", [TLOC, ODP], bf16, kind="ExternalOutput")
    rbufA = nc.dram_tensor("rbufA", [NSLOT * SLOTSZ], bf16,
                           kind="ExternalInput")
    rbufB = nc.dram_tensor("rbufB", [NSLOT * SLOTSZ], bf16,
                           kind="ExternalInput")
    wq = nc.dram_tensor("wq", [D, D], bf16, kind="ExternalInput")
    wk = nc.dram_tensor("wk", [D, D], bf16, kind="ExternalInput")
    wv = nc.dram_tensor("wv", [D, D], bf16, kind="ExternalInput")
    seTd = nc.dram_tensor("seTd", [HD, NH * NF], bf16, kind="ExternalInput")
    halod = nc.dram_tensor("halod", [128, 1], f32, kind="ExternalInput")
    identd = nc.dram_tensor("identd", [128, 128], f32, kind="ExternalInput")

    outers = _chunks_of(TLOC, OUTER)

    with tile.TileContext(nc) as tc, ExitStack() as ctx:
        const = ctx.enter_context(tc.tile_pool(name="const", bufs=1))
        wpool = ctx.enter_context(tc.tile_pool(name="wpool", bufs=1))
        xpool = ctx.enter_context(tc.tile_pool(name="xpool", bufs=2))
        qkp = ctx.enter_context(tc.tile_pool(name="qkp", bufs=2))
        vp = ctx.enter_context(tc.tile_pool(name="vp", bufs=2))
        bandp = ctx.enter_context(tc.tile_pool(name="bandp", bufs=4))
        bandf = ctx.enter_context(tc.tile_pool(name="bandf", bufs=4))
        tsp = ctx.enter_context(tc.tile_pool(name="tsp", bufs=2))
        outp = ctx.enter_context(tc.tile_pool(name="outp", bufs=2))
        psA = ctx.enter_context(tc.tile_pool(name="psA", bufs=2, space="PSUM"))
        psS = ctx.enter_context(tc.tile_pool(name="psS", bufs=2, space="PSUM"))
        psV = ctx.enter_context(tc.tile_pool(name="psV", bufs=4, space="PSUM"))

        # ---- constants / one-time init ----
        seT = const.tile([128, NH * NF], bf16, tag="seT")
        nc.sync.dma_start(seT[:], seTd[:, :])
        halo_sb = const.tile([128, 1], f32, tag="halo")
        nc.sync.dma_start(halo_sb[:], halod[:, :])
        ident = const.tile([128, 128], f32, tag="ident")
        nc.sync.dma_start(ident[:], identd[:, :])
        # four persistent skew-staging tiles (one per subchunk slot, two
        # 513-col planes each), poisoned once: the per-chunk rel scatter
        # only ever rewrites [0:13] of each 128-element segment, so the
        # poison in 13..127 (and col 512) survives; DRAM gap columns are
        # poisoned host-side (rbufA/rbufB arrive pre-filled with -1e9)
        pbands = [const.tile([128, 2 * RDP], bf16, tag=f"pband{i}",
                             name=f"pband{i}") for i in range(4)]
        for pb in pbands:
            nc.gpsimd.memset(pb[:, :], NEG)
        # Load order on the (FIFO) SP DMA queue is chosen so the PE can
        # start projecting as early as possible: wq interleaved with the
        # first outer's x tiles, then wk, wv.
        w_sb = {}
        xts0 = []
        kw0 = min(OUTER, TLOC) + HALO
        for dc in range(8):
            t = wpool.tile([128, D], bf16, tag=f"wq{dc}", name=f"wq{dc}")
            nc.sync.dma_start(t[:], wq[dc * 128:(dc + 1) * 128, :])
            w_sb[("q", dc)] = t
            xt = xpool.tile([128, OUTER + HALO], bf16, tag=f"xt{dc}")
            nc.sync.dma_start(xt[:, 0:kw0], xT[dc * 128:(dc + 1) * 128, 0:kw0])
            xts0.append(xt)
        for name, w in (("k", wk), ("v", wv)):
            for dc in range(8):
                t = wpool.tile([128, D], bf16, tag=f"w{name}{dc}",
                               name=f"w{name}{dc}")
                nc.sync.dma_start(t[:], w[dc * 128:(dc + 1) * 128, :])
                w_sb[(name, dc)] = t

        # deferred PV tails (see the attention loop below)
        pending = []

        def _emit_tail(p):
            mw, bw, c0 = p["mw"], p["bw"], p["c0"]
            pext, vt = p["pext"], p["vt"]
            pvb = [psV.tile([128, 512], f32, tag="pv", name=f"pvb{b}")
                   for b in range(3)]
            for n in range(NH):
                nc.tensor.matmul(
                    pvb[n // 3][0:mw,
                                (n % 3) * (HD + 1):(n % 3) * (HD + 1) + HD + 1],
                    pext[0:bw, n, 0:mw],
                    vt[0:bw, n * (HD + 1):(n + 1) * (HD + 1)],
                    start=True, stop=True)
            rec = tsp.tile([128, NH], f32, tag="rec")
            out_sb = outp.tile([128, D], bf16, tag="osb")
            for b in range(3):
                nsl = 3 if b < 2 else 2
                pb = pvb[b]
                nc.vector.reciprocal(
                    rec[0:mw, 3 * b:3 * b + nsl],
                    bass.AP(pb.tensor, pb.offset + HD,
                            [[pb.ap[0][0], mw], [HD + 1, nsl]]))
                nc.vector.tensor_mul(
                    out_sb[0:mw, 3 * b * HD:(3 * b + nsl) * HD],
                    bass.AP(pb.tensor, pb.offset,
                            [[pb.ap[0][0], mw], [HD + 1, nsl], [1, HD]]),
                    bass.AP(rec.tensor, rec.offset + 3 * b,
                            [[rec.ap[0][0], mw], [1, nsl], [0, HD]]))
            # two 1KB-row strided stores (sub-2KB rows spray all 16 engines)
            for h in range(2):
                nc.gpsimd.dma_start(
                    bass.AP(out, c0 * ODP + h * (D // 2),
                            [[ODP, mw], [1, D // 2]]),
                    out_sb[0:mw, h * (D // 2):(h + 1) * (D // 2)])

        # ---- main loop over outer chunks ----
        xts_next = xts0
        for oi, (t0, ow) in enumerate(outers):
            kw = ow + HALO
            subs = _chunks_of(ow, MW)
            xts = xts_next

            # Q projections for all heads first (their weights arrive
            # first); rel logits r[q, F] interleave per head into one psum
            # bank, then get scattered (f32->bf16) into poisoned staging
            # rows and round tripped through two independent DRAM skew
            # planes (4 heads each): the skewed re-read returns
            # band[q, n, j] = rel value or -1e9, i.e. the rel term plus the
            # causal/window mask in one tile
            rel_ps = psS.tile([128, 512], f32, tag="st", name="rel_ps")
            QT, KT = [], []
            for n in range(NH):
                qt = qkp.tile([128, OUTER], bf16, tag=f"qt{n}")
                pq = psA.tile([128, 512], f32, tag="a", name="pq")
                for dc in range(8):
                    nc.tensor.matmul(pq[:, 0:ow],
                                     w_sb[("q", dc)][:, n * HD:(n + 1) * HD],
                                     xts[dc][:, HALO:HALO + ow],
                                     start=(dc == 0), stop=(dc == 7))
                nc.vector.tensor_copy(qt[:, 0:ow], pq[:, 0:ow])
                QT.append(qt)
                # rel logits for this head immediately: each skew plane's
                # scatter+write can then start as soon as its 4 heads of rel
                # are done (plane A after head 3, plane B after head 7)
                for si, (c0l, mw) in enumerate(subs):
                    nc.tensor.matmul(
                        rel_ps[0:mw, si * 104 + n * NF:si * 104 + (n + 1) * NF],
                        qt[:, c0l:c0l + mw], seT[:, n * NF:(n + 1) * NF],
                        start=True, stop=True)
                if n == 2 and pending:
                    # flush the previous outer's last PV tail once the PE
                    # queue has fresh projection work ahead of it
                    _emit_tail(pending.pop(0))

            # per subchunk: scatter both planes (DVE), write each plane and
            # read it back skewed via gpsimd SWDGE; the two planes live in
            # separate DRAM tensors so each read only waits its own write
            for si, (c0l, mw) in enumerate(subs):
                ci = (t0 // OUTER) * 4 + si
                slot = (ci % NSLOT) * SLOTSZ
                pb = pbands[si]
                for pl, rb in ((0, rbufA), (1, rbufB)):
                    nc.vector.tensor_copy(
                        bass.AP(pb.tensor, pb.offset + pl * RDP,
                                [[pb.ap[0][0], mw], [128, 4], [1, NF]]),
                        bass.AP(rel_ps.tensor,
                                rel_ps.offset + si * 104 + pl * 4 * NF,
                                [[rel_ps.ap[0][0], mw], [NF, 4], [1, NF]]))
                    nc.gpsimd.dma_start(
                        bass.AP(rb, slot, [[ROWP, mw], [1, RDP]]),
                        pb[0:mw, pl * RDP:(pl + 1) * RDP])
            bands = []
            for si, (c0l, mw) in enumerate(subs):
                ci = (t0 // OUTER) * 4 + si
                slot = (ci % NSLOT) * SLOTSZ
                bw = mw + HALO
                bandb = bandp.tile([128, NH, 128], bf16, tag="bandb")
                for pl, rb in ((0, rbufA), (1, rbufB)):
                    nc.gpsimd.dma_start(
                        bandb[0:mw, 4 * pl:4 * pl + 4, 0:bw],
                        bass.AP(rb, slot, [[RDP, mw], [128, 4], [1, bw]]))
                bands.append(bandb)
            # prefetch the next outer's x tiles (sync queue, behind the
            # band reads) so the next Q projections never wait on DMA
            if oi + 1 < len(outers):
                nt0, now_ = outers[oi + 1]
                nkw = now_ + HALO
                xts_next = []
                for dc in range(8):
                    xt = xpool.tile([128, OUTER + HALO], bf16, tag=f"xt{dc}")
                    nc.sync.dma_start(
                        xt[:, 0:nkw],
                        xT[dc * 128:(dc + 1) * 128, nt0:nt0 + nkw])
                    xts_next.append(xt)

            # K projections for all heads
            for n in range(NH):
                kt = qkp.tile([128, OUTER + HALO], bf16, tag=f"kt{n}")
                pk = psA.tile([128, 512], f32, tag="a", name="pk")
                for dc in range(8):
                    nc.tensor.matmul(pk[:, 0:kw],
                                     w_sb[("k", dc)][:, n * HD:(n + 1) * HD],
                                     xts[dc][:, 0:kw],
                                     start=(dc == 0), stop=(dc == 7))
                nc.scalar.copy(kt[:, 0:kw], pk[:, 0:kw])
                KT.append(kt)

            # ---- attention subchunks (software-pipelined: the PV tail of
            # chunk c is emitted after chunk c+1's head so the in-order PE
            # queue has work while ACT produces exp(c)) ----
            for si, (c0l, mw) in enumerate(subs):
                c0 = t0 + c0l
                bw = mw + HALO
                bandb = bands[si]

                # V first (PE work that needs no band data): the band
                # roundtrip gets the whole V phase as extra latency cover
                # before the first transpose consumes it
                vt = vp.tile([128, NH * (HD + 1)], bf16, tag="vt")
                vt3 = vt.rearrange("p (a b) -> p a b", a=NH)
                for hh in range(2):
                    pvv = psV.tile([128, 512], f32, tag="pv", name="pvv")
                    for dc in range(8):
                        nc.tensor.matmul(
                            pvv[0:bw, 0:512], xts[dc][:, c0l:c0l + bw],
                            w_sb[("v", dc)][:, hh * 512:(hh + 1) * 512],
                            start=(dc == 0), stop=(dc == 7))
                    nc.vector.tensor_copy(vt3[0:bw, hh * 4:(hh + 1) * 4, 0:HD],
                                          pvv[0:bw, 0:512])
                nc.gpsimd.memset(vt3[0:bw, :, HD:HD + 1], 1.0)

                # upcast the bf16 band to f32 per plane AFTER the vt copies:
                # it is consumed only by the transposes below, and keeping it
                # out of the DVE queue ahead of the vt copies stops PV tails
                # (whose PSUM banks rotate onto vt-copy consumers) from
                # transitively waiting on band-read DMA latency
                band = bandf.tile([128, NH, 128], f32, tag="band")
                for pl in range(2):
                    nc.vector.tensor_copy(band[0:mw, 4 * pl:4 * pl + 4, 0:bw],
                                          bandb[0:mw, 4 * pl:4 * pl + 4, 0:bw])

                if t0 == 0 and si == 0:
                    # global-start halo: keys j<12 are zero padding on
                    # first-half cores (halod = -1e9 there, 0 elsewhere)
                    nc.vector.tensor_scalar_add(
                        band[0:mw, :, 0:HALO], band[0:mw, :, 0:HALO],
                        halo_sb[0:mw, :])

                # S^T[j, q] = k_j . q_q per head, then the band tile (rel
                # term + mask, natural [q, j] orientation) is transposed by
                # the PE into the same accumulation group; start/stop pairs
                # stay adjacent per bank (start=True resets the whole bank's
                # accumulation group)
                st = [psS.tile([128, 512], f32, tag="st", name=f"st{i}")
                      for i in range(2)]
                for n in range(NH):
                    nc.tensor.matmul(
                        st[n // 4][0:bw, (n % 4) * MW:(n % 4) * MW + mw],
                        KT[n][:, c0l:c0l + bw], QT[n][:, c0l:c0l + mw],
                        start=True, stop=False)
                    nc.tensor.matmul(
                        st[n // 4][0:bw, (n % 4) * MW:(n % 4) * MW + mw],
                        band[0:mw, n, 0:bw], ident[0:mw, 0:mw],
                        is_transpose=True, start=False, stop=True)

                th = tsp.tile([128, NH, MW], f32, tag="th")
                for i in range(2):
                    nc.scalar.activation(
                        th[0:bw, i * 4:(i + 1) * 4, 0:mw],
                        bass.AP(st[i].tensor, st[i].offset,
                                [[st[i].ap[0][0], bw], [MW, 4], [1, mw]]),
                        AF.Tanh, scale=1.0 / CAP)
                pext = tsp.tile([128, NH, MW], bf16, tag="pext")
                nc.scalar.activation(pext[0:bw, :, 0:mw], th[0:bw, :, 0:mw],
                                     AF.Exp, scale=CAP)

                pending.append({"mw": mw, "bw": bw, "c0": c0,
                                "pext": pext, "vt": vt})
                if len(pending) > 1:
                    _emit_tail(pending.pop(0))
            # leave the final tail of this outer pending; it is flushed
            # early in the next outer's projection phase (or at the end)

        while pending:
            _emit_tail(pending.pop(0))

    _split_excess_waits(nc, mybir)
    return nc


def _split_excess_waits(nc, mybir, limit=1):
    """walrus rejects >2 sync waits on one instruction; hoist the excess
    onto same-engine NoOps inserted just before (engine queues are in-order,
    so waits on the NoOp happen-before the original instruction)."""
    nid = [0]
    for f in nc.m.functions:
        for blk in f.blocks:
            new = []
            for ins in blk.instructions:
                si = ins.sync_info
                if si is not None and si.on_wait and len(si.on_wait) > limit:
                    waits = list(si.on_wait)
                    keep = waits[-limit:]
                    rest = waits[:-limit]
                    while rest:
                        take, rest = rest[:limit], rest[limit:]
                        nop = mybir.InstNoOp(
                            name=f"waitnop-{nid[0]}", engine=ins.engine,
                            ins=[], outs=[])
                        nid[0] += 1
                        nop.sync_info = mybir.SyncInfo(
                            on_wait=take, on_update=[])
                        new.append(nop)
                    ins.sync_info = mybir.SyncInfo(
                        on_wait=keep, on_update=list(si.on_update))
                new.append(ins)
            blk.instructions[:] = new


_NC_CACHE = None


def _get_program():
    global _NC_CACHE
    if _NC_CACHE is None:
        _NC_CACHE = _build_program()
    return _NC_CACHE


def _host_inputs(x, mask, causal_valid_mask, wq, wk, wv, pos_proj):
    x = np.asarray(x, np.float32)
    wq = np.asarray(wq, np.float32) * QSC   # fold q scaling into wq
    wk = np.asarray(wk, np.float32)
    wv = np.asarray(wv, np.float32)
    pp = np.asarray(pos_proj, np.float32)   # [D, N, H]

    # constant sinusoidal table projected through pos_proj (input-independent)
    pos = np.arange(HALO, -1, -1, dtype=np.float32)  # [13]: 12..0
    nt = D // 2
    inc = np.log(10000.0) / (nt - 1)
    inv = np.exp(np.arange(nt, dtype=np.float32) * -inc)
    st = pos[:, None] * inv[None, :]
    sin_emb = np.concatenate([np.sin(st), np.cos(st)], axis=1)  # [13, D]
    # seT[h, n*13+F] = sum_d sin_emb[F, d] * pos_proj[d, n, h]
    se = np.einsum("fd,dnh->hnf", sin_emb, pp)       # [H, N, F]
    seTh = np.ascontiguousarray(se.reshape(HD, NH * NF)).astype(BF)

    wqb = wq.astype(BF)
    wkb = wk.astype(BF)
    wvb = wv.astype(BF)

    rpois = np.full(NSLOT * SLOTSZ, NEG, dtype=np.float32).astype(BF)
    in_maps = []
    for core in range(8):
        b, half = divmod(core, 2)
        t0 = half * TLOC
        lo = t0 - HALO
        if lo < 0:
            xs = np.concatenate(
                [np.zeros((HALO, D), np.float32), x[b, 0:t0 + TLOC]], axis=0)
        else:
            xs = x[b, lo:t0 + TLOC]
        halo = np.full((128, 1), NEG if half == 0 else 0.0, np.float32)
        in_maps.append({
            "xT": np.ascontiguousarray(xs.T).astype(BF),
            "wq": wqb, "wk": wkb, "wv": wvb,
            "seTd": seTh, "halod": halo,
            "identd": np.eye(128, dtype=np.float32),
            "rbufA": rpois, "rbufB": rpois,
        })
    return in_maps


_LAST_RESULTS = {"exec_time_ns": None}


def kernel(x, mask, causal_valid_mask, wq, wk, wv, pos_proj, _trace=False):
    from concourse import bass_utils
    nc = _get_program()
    in_maps = _host_inputs(x, mask, causal_valid_mask, wq, wk, wv, pos_proj)
    res = bass_utils.run_bass_kernel_spmd(
        nc, in_maps, core_ids=list(range(8)), trace=_trace)
    _LAST_RESULTS["exec_time_ns"] = res.exec_time_ns
    _LAST_RESULTS["profile_json"] = getattr(res, "profile_json", None)
    outs = [res.results[c]["out"] for c in range(8)]
    full = np.empty((B, T, NH, HD), np.float32)
    for core in range(8):
        b, half = divmod(core, 2)
        t0 = half * TLOC
        full[b, t0:t0 + TLOC] = np.asarray(
            outs[core][:, :D], dtype=np.float32).reshape(TLOC, NH, HD)
    return full


# revision 44
# speedup vs baseline: 1.0110x; 1.0110x over previous
"""Trainium2 Bass kernel for nn_AttentionBlock (blockwise local attention).

Per batch and head this is sliding-window causal attention with window 13
(query p attends to keys p-12..p), plus a relative-position logit term,
tanh soft-capping at 50, and key-validity masking.

Sharding: 8 cores = 4 batches x 2 T-halves. Each core computes all 8 heads
for 4080 queries of one batch half (12-row K/V halo) from host-pretransposed
x^T and the projection weights (q-scale folded into wq on host).

Per-core pipeline (bf16 matmuls, fp32 accumulation), chunked 116 queries at
a time so each chunk's 128-wide key band fits the PSUM partition dim:
  - Q^T,K^T = W^T @ x^T per head (PE); V in natural [key, head] layout (PE);
    rel logits r[q,F]=q_q.se_F (PE) interleaved per head into one psum bank
  - rel logits are scattered (DVE, f32->bf16) into persistent -1e9-poisoned
    SBUF row tiles and round-tripped through two independent DRAM skew
    planes (4 heads each, row pitch 514 written / 513 read back): the
    pitch-shifted re-read returns band[q,n,j] = rel value when
    0 <= q-j < 13 and -1e9 otherwise, i.e. the rel-position term plus the
    causal/window mask in one bf16 tile; separate DRAM tensors per plane
    keep each read gated only on its own plane's write
  - per subchunk: V matmuls first (band-latency cover), then per head
    S^T[j,q] = k_j.q_q into packed PSUM (4 heads/bank) with the upcast
    band tile transposed by the PE into the same accumulation group
    (start/stop pairs stay adjacent: start resets the bank's group)
  - tanh, exp (ACT, fused across heads) -> P^T in SBUF, bf16
  - P^T @ [V | 1] per head accumulates numerator and denominator (PE,
    deferred one subchunk so the in-order PE queue stays fed); reciprocal +
    broadcast multiply (DVE, bf16 out) -> two 1KB-row strided stores

Engine-queue discipline (the real bottlenecks were in-order queues and
DMA-engine assignment, not bandwidth): qt copies on DVE, kt copies on ACT,
x/weight loads on the SP HWDGE queue, and the whole band roundtrip plus
the output stores issued via gpsimd SWDGE -- SWDGE traffic spreads across
all 16 DMA engines (HWDGE pinned rbuf/out traffic to 4) and keeps DMA
issue off the ACT/SP queues. The bf16->f32 band upcast runs late (in the
attention loop) so in-order DVE progress never couples PV tails to
band-read DMA latency. PSUM pools are split (projections 2 / S+rel 2 /
V+PV 4 banks) so bank rotation never makes a projection wait on attention
consumers.
"""
import sys
import numpy as np

sys.path.insert(0, "/opt/trn_rl_repo")
import ml_dtypes  # noqa: E402

BF = ml_dtypes.bfloat16

B, T, D = 4, 8160, 1024
NH, HD = 8, 128
HALO = 12
CAP = 50.0
QSC = 1.0 / np.sqrt(HD)

TLOC = T // 2         # 4080 queries per core
KLOC = TLOC + HALO    # 4092
OUTER = 464           # queries per outer chunk (4 subchunks of 116)
MW = 116              # queries per attention subchunk (band = 116+12 = 128)
NF = 13               # relative-position offsets
ROWP = 4 * 128 + 2    # 514: skew-plane row pitch (4 heads per plane); the
                      # extra poison+gap cols keep the DRAM side strided so
                      # the DGE sprays all 16 engines
RDP = ROWP - 1        # 513: skewed read row pitch
ODP = D + 8           # 1032: padded out row pitch (strided store, 16 engines)
NSLOT = 8             # ring slots per plane (one per in-flight subchunk)
SLOTSZ = MW * ROWP    # bf16 elements per ring slot
NEG = -1.0e9


def _chunks_of(total, size):
    out = []
    o = 0
    while o < total:
        out.append((o, min(size, total - o)))
        o += size
    return out


def _build_program():
    import concourse.bass as bass
    import concourse.tile as tile
    from concourse import mybir
    from contextlib import ExitStack

    f32 = mybir.dt.float32
    bf16 = mybir.dt.bfloat16
    AF = mybir.ActivationFunctionType

    nc = bass.Bass(target_bir_lowering=False, debug=False)

    xT = nc.dram_tensor("xT", [D, KLOC], bf16, kind="ExternalInput")
    out = nc.dram_tensor("out", [TLOC, ODP], bf16, kind="ExternalOutput")
    rbufA = nc.dram_tensor("rbufA", [NSLOT * SLOTSZ], bf16,
                           kind="ExternalInput")
    rbufB = nc.dram_tensor("rbufB", [NSLOT * SLOTSZ], bf16,
                           kind="ExternalInput")
    wq = nc.dram_tensor("wq", [D, D], bf16, kind="ExternalInput")
    wk = nc.dram_tensor("wk", [D, D], bf16, kind="ExternalInput")
    wv = nc.dram_tensor("wv", [D, D], bf16, kind="ExternalInput")
    seTd = nc.dram_tensor("seTd", [HD, NH * NF], bf16, kind="ExternalInput")
    halod = nc.dram_tensor("halod", [128, 1], f32, kind="ExternalInput")
    identd = nc.dram_tensor("identd", [128, 128], f32, kind="ExternalInput")

    outers = _chunks_of(TLOC, OUTER)

    with tile.TileContext(nc) as tc, ExitStack() as ctx:
        const = ctx.enter_context(tc.tile_pool(name="const", bufs=1))
        wpool = ctx.enter_context(tc.tile_pool(name="wpool", bufs=1))
        xpool = ctx.enter_context(tc.tile_pool(name="xpool", bufs=2))
        qkp = ctx.enter_context(tc.tile_pool(name="qkp", bufs=2))
        vp = ctx.enter_context(tc.tile_pool(name="vp", bufs=2))
        bandp = ctx.enter_context(tc.tile_pool(name="bandp", bufs=4))
        bandf = ctx.enter_context(tc.tile_pool(name="bandf", bufs=4))
        tsp = ctx.enter_context(tc.tile_pool(name="tsp", bufs=2))
        outp = ctx.enter_context(tc.tile_pool(name="outp", bufs=2))
        psA = ctx.enter_context(tc.tile_pool(name="psA", bufs=2, space="PSUM"))
        psS = ctx.enter_context(tc.tile_pool(name="psS", bufs=2, space="PSUM"))
        psV = ctx.enter_context(tc.tile_pool(name="psV", bufs=4, space="PSUM"))

        # ---- constants / one-time init ----
        seT = const.tile([128, NH * NF], bf16, tag="seT")
        nc.sync.dma_start(seT[:], seTd[:, :])
        halo_sb = const.tile([128, 1], f32, tag="halo")
        nc.sync.dma_start(halo_sb[:], halod[:, :])
        ident = const.tile([128, 128], f32, tag="ident")
        nc.sync.dma_start(ident[:], identd[:, :])
        # four persistent skew-staging tiles (one per subchunk slot, two
        # 513-col planes each), poisoned once: the per-chunk rel scatter
        # only ever rewrites [0:13] of each 128-element segment, so the
        # poison in 13..127 (and col 512) survives; DRAM gap columns are
        # poisoned host-side (rbufA/rbufB arrive pre-filled with -1e9)
        pbands = [const.tile([128, 2 * RDP], bf16, tag=f"pband{i}",
                             name=f"pband{i}") for i in range(4)]
        for pb in pbands:
            nc.gpsimd.memset(pb[:, :], NEG)
        # Load order on the (FIFO) SP DMA queue is chosen so the PE can
        # start projecting as early as possible: wq interleaved with the
        # first outer's x tiles, then wk, wv.
        w_sb = {}
        xts0 = []
        kw0 = min(OUTER, TLOC) + HALO
        for dc in range(8):
            t = wpool.tile([128, D], bf16, tag=f"wq{dc}", name=f"wq{dc}")
            nc.sync.dma_start(t[:], wq[dc * 128:(dc + 1) * 128, :])
            w_sb[("q", dc)] = t
            xt = xpool.tile([128, OUTER + HALO], bf16, tag=f"xt{dc}")
            nc.sync.dma_start(xt[:, 0:kw0], xT[dc * 128:(dc + 1) * 128, 0:kw0])
            xts0.append(xt)
        # wk/wv issue on the other queues so all three load streams start
        # immediately (wq+x0 are the critical path; wk/wv arrive later)
        for name, w, eng in (("k", wk, nc.scalar), ("v", wv, nc.gpsimd)):
            for dc in range(8):
                t = wpool.tile([128, D], bf16, tag=f"w{name}{dc}",
                               name=f"w{name}{dc}")
                eng.dma_start(t[:], w[dc * 128:(dc + 1) * 128, :])
                w_sb[(name, dc)] = t

        # deferred PV tails (see the attention loop below)
        pending = []

        def _emit_tail(p):
            mw, bw, c0 = p["mw"], p["bw"], p["c0"]
            pext, vt = p["pext"], p["vt"]
            pvb = [psV.tile([128, 512], f32, tag="pv", name=f"pvb{b}")
                   for b in range(3)]
            for n in range(NH):
                nc.tensor.matmul(
                    pvb[n // 3][0:mw,
                                (n % 3) * (HD + 1):(n % 3) * (HD + 1) + HD + 1],
                    pext[0:bw, n, 0:mw],
                    vt[0:bw, n * (HD + 1):(n + 1) * (HD + 1)],
                    start=True, stop=True)
            rec = tsp.tile([128, NH], f32, tag="rec")
            out_sb = outp.tile([128, D], bf16, tag="osb")
            for b in range(3):
                nsl = 3 if b < 2 else 2
                pb = pvb[b]
                nc.vector.reciprocal(
                    rec[0:mw, 3 * b:3 * b + nsl],
                    bass.AP(pb.tensor, pb.offset + HD,
                            [[pb.ap[0][0], mw], [HD + 1, nsl]]))
                nc.vector.tensor_mul(
                    out_sb[0:mw, 3 * b * HD:(3 * b + nsl) * HD],
                    bass.AP(pb.tensor, pb.offset,
                            [[pb.ap[0][0], mw], [HD + 1, nsl], [1, HD]]),
                    bass.AP(rec.tensor, rec.offset + 3 * b,
                            [[rec.ap[0][0], mw], [1, nsl], [0, HD]]))
            # two 1KB-row strided stores (sub-2KB rows spray all 16 engines)
            for h in range(2):
                nc.gpsimd.dma_start(
                    bass.AP(out, c0 * ODP + h * (D // 2),
                            [[ODP, mw], [1, D // 2]]),
                    out_sb[0:mw, h * (D // 2):(h + 1) * (D // 2)])

        # ---- main loop over outer chunks ----
        xts_next = xts0
        for oi, (t0, ow) in enumerate(outers):
            kw = ow + HALO
            subs = _chunks_of(ow, MW)
            xts = xts_next

            # Q projections for all heads first (their weights arrive
            # first); rel logits r[q, F] interleave per head into one psum
            # bank, then get scattered (f32->bf16) into poisoned staging
            # rows and round tripped through two independent DRAM skew
            # planes (4 heads each): the skewed re-read returns
            # band[q, n, j] = rel value or -1e9, i.e. the rel term plus the
            # causal/window mask in one tile
            rel_ps = psS.tile([128, 512], f32, tag="st", name="rel_ps")
            QT, KT = [], []
            for n in range(NH):
                qt = qkp.tile([128, OUTER], bf16, tag=f"qt{n}")
                pq = psA.tile([128, 512], f32, tag="a", name="pq")
                for dc in range(8):
                    nc.tensor.matmul(pq[:, 0:ow],
                                     w_sb[("q", dc)][:, n * HD:(n + 1) * HD],
                                     xts[dc][:, HALO:HALO + ow],
                                     start=(dc == 0), stop=(dc == 7))
                nc.vector.tensor_copy(qt[:, 0:ow], pq[:, 0:ow])
                QT.append(qt)
                # rel logits for this head immediately: each skew plane's
                # scatter+write can then start as soon as its 4 heads of rel
                # are done (plane A after head 3, plane B after head 7)
                for si, (c0l, mw) in enumerate(subs):
                    nc.tensor.matmul(
                        rel_ps[0:mw, si * 104 + n * NF:si * 104 + (n + 1) * NF],
                        qt[:, c0l:c0l + mw], seT[:, n * NF:(n + 1) * NF],
                        start=True, stop=True)
                if n == 2 and pending:
                    # flush the previous outer's last PV tail once the PE
                    # queue has fresh projection work ahead of it
                    _emit_tail(pending.pop(0))

            # per subchunk: scatter both planes (DVE), write each plane and
            # read it back skewed via gpsimd SWDGE; the two planes live in
            # separate DRAM tensors so each read only waits its own write
            for si, (c0l, mw) in enumerate(subs):
                ci = (t0 // OUTER) * 4 + si
                slot = (ci % NSLOT) * SLOTSZ
                pb = pbands[si]
                for pl, rb in ((0, rbufA), (1, rbufB)):
                    nc.vector.tensor_copy(
                        bass.AP(pb.tensor, pb.offset + pl * RDP,
                                [[pb.ap[0][0], mw], [128, 4], [1, NF]]),
                        bass.AP(rel_ps.tensor,
                                rel_ps.offset + si * 104 + pl * 4 * NF,
                                [[rel_ps.ap[0][0], mw], [NF, 4], [1, NF]]))
                    nc.gpsimd.dma_start(
                        bass.AP(rb, slot, [[ROWP, mw], [1, RDP]]),
                        pb[0:mw, pl * RDP:(pl + 1) * RDP])
            bands = []
            for si, (c0l, mw) in enumerate(subs):
                ci = (t0 // OUTER) * 4 + si
                slot = (ci % NSLOT) * SLOTSZ
                bw = mw + HALO
                bandb = bandp.tile([128, NH, 128], bf16, tag="bandb")
                for pl, rb in ((0, rbufA), (1, rbufB)):
                    nc.gpsimd.dma_start(
                        bandb[0:mw, 4 * pl:4 * pl + 4, 0:bw],
                        bass.AP(rb, slot, [[RDP, mw], [128, 4], [1, bw]]))
                bands.append(bandb)
            # prefetch the next outer's x tiles (sync queue, behind the
            # band reads) so the next Q projections never wait on DMA
            if oi + 1 < len(outers):
                nt0, now_ = outers[oi + 1]
                nkw = now_ + HALO
                xts_next = []
                for dc in range(8):
                    xt = xpool.tile([128, OUTER + HALO], bf16, tag=f"xt{dc}")
                    nc.sync.dma_start(
                        xt[:, 0:nkw],
                        xT[dc * 128:(dc + 1) * 128, nt0:nt0 + nkw])
                    xts_next.append(xt)

            # K projections for all heads
            for n in range(NH):
                kt = qkp.tile([128, OUTER + HALO], bf16, tag=f"kt{n}")
                pk = psA.tile([128, 512], f32, tag="a", name="pk")
                for dc in range(8):
                    nc.tensor.matmul(pk[:, 0:kw],
                                     w_sb[("k", dc)][:, n * HD:(n + 1) * HD],
                                     xts[dc][:, 0:kw],
                                     start=(dc == 0), stop=(dc == 7))
                nc.scalar.copy(kt[:, 0:kw], pk[:, 0:kw])
                KT.append(kt)

            # ---- attention subchunks (software-pipelined: the PV tail of
            # chunk c is emitted after chunk c+1's head so the in-order PE
            # queue has work while ACT produces exp(c)) ----
            for si, (c0l, mw) in enumerate(subs):
                c0 = t0 + c0l
                bw = mw + HALO
                bandb = bands[si]

                # V first (PE work that needs no band data): the band
                # roundtrip gets the whole V phase as extra latency cover
                # before the first transpose consumes it
                vt = vp.tile([128, NH * (HD + 1)], bf16, tag="vt")
                vt3 = vt.rearrange("p (a b) -> p a b", a=NH)
                for hh in range(2):
                    pvv = psV.tile([128, 512], f32, tag="pv", name="pvv")
                    for dc in range(8):
                        nc.tensor.matmul(
                            pvv[0:bw, 0:512], xts[dc][:, c0l:c0l + bw],
                            w_sb[("v", dc)][:, hh * 512:(hh + 1) * 512],
                            start=(dc == 0), stop=(dc == 7))
                    nc.vector.tensor_copy(vt3[0:bw, hh * 4:(hh + 1) * 4, 0:HD],
                                          pvv[0:bw, 0:512])
                nc.gpsimd.memset(vt3[0:bw, :, HD:HD + 1], 1.0)

                # upcast the bf16 band to f32 per plane AFTER the vt copies:
                # it is consumed only by the transposes below, and keeping it
                # out of the DVE queue ahead of the vt copies stops PV tails
                # (whose PSUM banks rotate onto vt-copy consumers) from
                # transitively waiting on band-read DMA latency
                band = bandf.tile([128, NH, 128], f32, tag="band")
                for pl in range(2):
                    nc.vector.tensor_copy(band[0:mw, 4 * pl:4 * pl + 4, 0:bw],
                                          bandb[0:mw, 4 * pl:4 * pl + 4, 0:bw])

                if t0 == 0 and si == 0:
                    # global-start halo: keys j<12 are zero padding on
                    # first-half cores (halod = -1e9 there, 0 elsewhere)
                    nc.vector.tensor_scalar_add(
                        band[0:mw, :, 0:HALO], band[0:mw, :, 0:HALO],
                        halo_sb[0:mw, :])

                # S^T[j, q] = k_j . q_q per head, then the band tile (rel
                # term + mask, natural [q, j] orientation) is transposed by
                # the PE into the same accumulation group; start/stop pairs
                # stay adjacent per bank (start=True resets the whole bank's
                # accumulation group)
                st = [psS.tile([128, 512], f32, tag="st", name=f"st{i}")
                      for i in range(2)]
                for n in range(NH):
                    nc.tensor.matmul(
                        st[n // 4][0:bw, (n % 4) * MW:(n % 4) * MW + mw],
                        KT[n][:, c0l:c0l + bw], QT[n][:, c0l:c0l + mw],
                        start=True, stop=False)
                    nc.tensor.matmul(
                        st[n // 4][0:bw, (n % 4) * MW:(n % 4) * MW + mw],
                        band[0:mw, n, 0:bw], ident[0:mw, 0:mw],
                        is_transpose=True, start=False, stop=True)

                th = tsp.tile([128, NH, MW], f32, tag="th")
                for i in range(2):
                    nc.scalar.activation(
                        th[0:bw, i * 4:(i + 1) * 4, 0:mw],
                        bass.AP(st[i].tensor, st[i].offset,
                                [[st[i].ap[0][0], bw], [MW, 4], [1, mw]]),
                        AF.Tanh, scale=1.0 / CAP)
                pext = tsp.tile([128, NH, MW], bf16, tag="pext")
                nc.scalar.activation(pext[0:bw, :, 0:mw], th[0:bw, :, 0:mw],
                                     AF.Exp, scale=CAP)

                pending.append({"mw": mw, "bw": bw, "c0": c0,
                                "pext": pext, "vt": vt})
                if len(pending) > 1:
                    _emit_tail(pending.pop(0))
            # leave the final tail of this outer pending; it is flushed
            # early in the next outer's projection phase (or at the end)

        while pending:
            _emit_tail(pending.pop(0))

    _split_excess_waits(nc, mybir)
    return nc


def _split_excess_waits(nc, mybir, limit=1):
    """walrus rejects >2 sync waits on one instruction; hoist the excess
    onto same-engine NoOps inserted just before (engine queues are in-order,
    so waits on the NoOp happen-before the original instruction)."""
    nid = [0]
    for f in nc.m.functions:
        for blk in f.blocks:
            new = []
            for ins in blk.instructions:
                si = ins.sync_info
                if si is not None and si.on_wait and len(si.on_wait) > limit:
                    waits = list(si.on_wait)
                    keep = waits[-limit:]
                    rest = waits[:-limit]
                    while rest:
                        take, rest = rest[:limit], rest[limit:]
                        nop = mybir.InstNoOp(
                            name=f"waitnop-{nid[0]}", engine=ins.engine,
                            ins=[], outs=[])
                        nid[0] += 1
                        nop.sync_info = mybir.SyncInfo(
                            on_wait=take, on_update=[])
                        new.append(nop)
                    ins.sync_info = mybir.SyncInfo(
                        on_wait=keep, on_update=list(si.on_update))
                new.append(ins)
            blk.instructions[:] = new


_NC_CACHE = None


def _get_program():
    global _NC_CACHE
    if _NC_CACHE is None:
        _NC_CACHE = _build_program()
    return _NC_CACHE


def _host_inputs(x, mask, causal_valid_mask, wq, wk, wv, pos_proj):
    x = np.asarray(x, np.float32)
    wq = np.asarray(wq, np.float32) * QSC   # fold q scaling into wq
    wk = np.asarray(wk, np.float32)
    wv = np.asarray(wv, np.float32)
    pp = np.asarray(pos_proj, np.float32)   # [D, N, H]

    # constant sinusoidal table projected through pos_proj (input-independent)
    pos = np.arange(HALO, -1, -1, dtype=np.float32)  # [13]: 12..0
    nt = D // 2
    inc = np.log(10000.0) / (nt - 1)
    inv = np.exp(np.arange(nt, dtype=np.float32) * -inc)
    st = pos[:, None] * inv[None, :]
    sin_emb = np.concatenate([np.sin(st), np.cos(st)], axis=1)  # [13, D]
    # seT[h, n*13+F] = sum_d sin_emb[F, d] * pos_proj[d, n, h]
    se = np.einsum("fd,dnh->hnf", sin_emb, pp)       # [H, N, F]
    seTh = np.ascontiguousarray(se.reshape(HD, NH * NF)).astype(BF)

    wqb = wq.astype(BF)
    wkb = wk.astype(BF)
    wvb = wv.astype(BF)

    rpois = np.full(NSLOT * SLOTSZ, NEG, dtype=np.float32).astype(BF)
    in_maps = []
    for core in range(8):
        b, half = divmod(core, 2)
        t0 = half * TLOC
        lo = t0 - HALO
        if lo < 0:
            xs = np.concatenate(
                [np.zeros((HALO, D), np.float32), x[b, 0:t0 + TLOC]], axis=0)
        else:
            xs = x[b, lo:t0 + TLOC]
        halo = np.full((128, 1), NEG if half == 0 else 0.0, np.float32)
        in_maps.append({
            "xT": np.ascontiguousarray(xs.T).astype(BF),
            "wq": wqb, "wk": wkb, "wv": wvb,
            "seTd": seTh, "halod": halo,
            "identd": np.eye(128, dtype=np.float32),
            "rbufA": rpois, "rbufB": rpois,
        })
    return in_maps


_LAST_RESULTS = {"exec_time_ns": None}


def kernel(x, mask, causal_valid_mask, wq, wk, wv, pos_proj, _trace=False):
    from concourse import bass_utils
    nc = _get_program()
    in_maps = _host_inputs(x, mask, causal_valid_mask, wq, wk, wv, pos_proj)
    res = bass_utils.run_bass_kernel_spmd(
        nc, in_maps, core_ids=list(range(8)), trace=_trace)
    _LAST_RESULTS["exec_time_ns"] = res.exec_time_ns
    _LAST_RESULTS["profile_json"] = getattr(res, "profile_json", None)
    outs = [res.results[c]["out"] for c in range(8)]
    full = np.empty((B, T, NH, HD), np.float32)
    for core in range(8):
        b, half = divmod(core, 2)
        t0 = half * TLOC
        full[b, t0:t0 + TLOC] = np.asarray(
            outs[core][:, :D], dtype=np.float32).reshape(TLOC, NH, HD)
    return full


# revision 45
# speedup vs baseline: 1.0367x; 1.0254x over previous
"""Trainium2 Bass kernel for nn_AttentionBlock (blockwise local attention).

Per batch and head this is sliding-window causal attention with window 13
(query p attends to keys p-12..p), plus a relative-position logit term,
tanh soft-capping at 50, and key-validity masking.

Sharding: 8 cores = 4 batches x 2 T-halves. Each core computes all 8 heads
for 4080 queries of one batch half (12-row K/V halo) from host-pretransposed
x^T and the projection weights (q-scale folded into wq on host).

Per-core pipeline (bf16 matmuls, fp32 accumulation), chunked 116 queries at
a time so each chunk's 128-wide key band fits the PSUM partition dim:
  - Q^T,K^T = W^T @ x^T per head (PE); V in natural [key, head] layout (PE);
    rel logits r[q,F]=q_q.se_F (PE) interleaved per head into one psum bank
  - rel logits are scattered (DVE, f32->bf16) into persistent -1e9-poisoned
    SBUF row tiles and round-tripped through two independent DRAM skew
    planes (4 heads each, row pitch 514 written / 513 read back): the
    pitch-shifted re-read returns band[q,n,j] = rel value when
    0 <= q-j < 13 and -1e9 otherwise, i.e. the rel-position term plus the
    causal/window mask in one bf16 tile; separate DRAM tensors per plane
    keep each read gated only on its own plane's write
  - per subchunk: V matmuls first (band-latency cover), then per head
    S^T[j,q] = k_j.q_q into packed PSUM (4 heads/bank) with the upcast
    band tile transposed by the PE into the same accumulation group
    (start/stop pairs stay adjacent: start resets the bank's group)
  - tanh, exp (ACT, fused across heads) -> P^T in SBUF, bf16
  - P^T @ [V | 1] per head accumulates numerator and denominator (PE,
    deferred one subchunk so the in-order PE queue stays fed); reciprocal +
    broadcast multiply (DVE, bf16 out) -> two 1KB-row strided stores

Engine-queue discipline (the real bottlenecks were in-order queues and
DMA-engine assignment, not bandwidth): qt copies on DVE, kt copies on ACT,
x/weight loads on the SP HWDGE queue, and the whole band roundtrip plus
the output stores issued via gpsimd SWDGE -- SWDGE traffic spreads across
all 16 DMA engines (HWDGE pinned rbuf/out traffic to 4) and keeps DMA
issue off the ACT/SP queues. The bf16->f32 band upcast runs late (in the
attention loop) so in-order DVE progress never couples PV tails to
band-read DMA latency. PSUM pools are split (projections 2 / S+rel 2 /
V+PV 4 banks) so bank rotation never makes a projection wait on attention
consumers.
"""
import sys
import numpy as np

sys.path.insert(0, "/opt/trn_rl_repo")
import ml_dtypes  # noqa: E402

BF = ml_dtypes.bfloat16

B, T, D = 4, 8160, 1024
NH, HD = 8, 128
HALO = 12
CAP = 50.0
QSC = 1.0 / np.sqrt(HD)

TLOC = T // 2         # 4080 queries per core
KLOC = TLOC + HALO    # 4092
OUTER = 464           # queries per outer chunk (4 subchunks of 116)
MW = 116              # queries per attention subchunk (band = 116+12 = 128)
NF = 13               # relative-position offsets
ROWP = 4 * 128 + 2    # 514: skew-plane row pitch (4 heads per plane); the
                      # extra poison+gap cols keep the DRAM side strided so
                      # the DGE sprays all 16 engines
RDP = ROWP - 1        # 513: skewed read row pitch
ODP = D + 8           # 1032: padded out row pitch (strided store, 16 engines)
NSLOT = 8             # ring slots per plane (one per in-flight subchunk)
SLOTSZ = MW * ROWP    # bf16 elements per ring slot
NEG = -1.0e9


def _chunks_of(total, size):
    out = []
    o = 0
    while o < total:
        out.append((o, min(size, total - o)))
        o += size
    return out


def _build_program():
    import concourse.bass as bass
    import concourse.tile as tile
    from concourse import mybir
    from contextlib import ExitStack

    f32 = mybir.dt.float32
    bf16 = mybir.dt.bfloat16
    AF = mybir.ActivationFunctionType

    nc = bass.Bass(target_bir_lowering=False, debug=False)

    xT = nc.dram_tensor("xT", [D, KLOC], bf16, kind="ExternalInput")
    out = nc.dram_tensor("out", [TLOC, ODP], bf16, kind="ExternalOutput")
    rbufA = nc.dram_tensor("rbufA", [NSLOT * SLOTSZ], bf16,
                           kind="ExternalInput")
    rbufB = nc.dram_tensor("rbufB", [NSLOT * SLOTSZ], bf16,
                           kind="ExternalInput")
    wq = nc.dram_tensor("wq", [D, D], bf16, kind="ExternalInput")
    wk = nc.dram_tensor("wk", [D, D], bf16, kind="ExternalInput")
    wv = nc.dram_tensor("wv", [D, D], bf16, kind="ExternalInput")
    seTd = nc.dram_tensor("seTd", [HD, NH * NF], bf16, kind="ExternalInput")
    halod = nc.dram_tensor("halod", [128, 1], f32, kind="ExternalInput")
    identd = nc.dram_tensor("identd", [128, 128], f32, kind="ExternalInput")

    outers = _chunks_of(TLOC, OUTER)

    with tile.TileContext(nc) as tc, ExitStack() as ctx:
        const = ctx.enter_context(tc.tile_pool(name="const", bufs=1))
        wpool = ctx.enter_context(tc.tile_pool(name="wpool", bufs=1))
        xpool = ctx.enter_context(tc.tile_pool(name="xpool", bufs=2))
        qkp = ctx.enter_context(tc.tile_pool(name="qkp", bufs=2))
        vp = ctx.enter_context(tc.tile_pool(name="vp", bufs=2))
        bandp = ctx.enter_context(tc.tile_pool(name="bandp", bufs=4))
        bandf = ctx.enter_context(tc.tile_pool(name="bandf", bufs=4))
        tsp = ctx.enter_context(tc.tile_pool(name="tsp", bufs=2))
        outp = ctx.enter_context(tc.tile_pool(name="outp", bufs=2))
        psA = ctx.enter_context(tc.tile_pool(name="psA", bufs=2, space="PSUM"))
        psS = ctx.enter_context(tc.tile_pool(name="psS", bufs=2, space="PSUM"))
        psV = ctx.enter_context(tc.tile_pool(name="psV", bufs=4, space="PSUM"))

        # ---- constants / one-time init ----
        seT = const.tile([128, NH * NF], bf16, tag="seT")
        nc.sync.dma_start(seT[:], seTd[:, :])
        halo_sb = const.tile([128, 1], f32, tag="halo")
        nc.sync.dma_start(halo_sb[:], halod[:, :])
        ident = const.tile([128, 128], f32, tag="ident")
        nc.sync.dma_start(ident[:], identd[:, :])
        # four persistent skew-staging tiles (one per subchunk slot, two
        # 513-col planes each), poisoned once: the per-chunk rel scatter
        # only ever rewrites [0:13] of each 128-element segment, so the
        # poison in 13..127 (and col 512) survives; DRAM gap columns are
        # poisoned host-side (rbufA/rbufB arrive pre-filled with -1e9)
        pbands = [const.tile([128, 2 * RDP], bf16, tag=f"pband{i}",
                             name=f"pband{i}") for i in range(4)]
        for pb in pbands:
            nc.gpsimd.memset(pb[:, :], NEG)
        # Load order on the (FIFO) SP DMA queue is chosen so the PE can
        # start projecting as early as possible: wq interleaved with the
        # first outer's x tiles, then wk, wv.
        w_sb = {}
        xts0 = []
        kw0 = min(OUTER, TLOC) + HALO
        for dc in range(8):
            t = wpool.tile([128, D], bf16, tag=f"wq{dc}", name=f"wq{dc}")
            nc.sync.dma_start(t[:], wq[dc * 128:(dc + 1) * 128, :])
            w_sb[("q", dc)] = t
            xt = xpool.tile([128, OUTER + HALO], bf16, tag=f"xt{dc}")
            nc.sync.dma_start(xt[:, 0:kw0], xT[dc * 128:(dc + 1) * 128, 0:kw0])
            xts0.append(xt)
        for name, w in (("k", wk), ("v", wv)):
            for dc in range(8):
                t = wpool.tile([128, D], bf16, tag=f"w{name}{dc}",
                               name=f"w{name}{dc}")
                nc.sync.dma_start(t[:], w[dc * 128:(dc + 1) * 128, :])
                w_sb[(name, dc)] = t

        # deferred PV tails (see the attention loop below)
        pending = []

        def _emit_tail(p):
            mw, bw, c0 = p["mw"], p["bw"], p["c0"]
            pext, vt = p["pext"], p["vt"]
            pvb = [psV.tile([128, 512], f32, tag="pv", name=f"pvb{b}")
                   for b in range(3)]
            for n in range(NH):
                nc.tensor.matmul(
                    pvb[n // 3][0:mw,
                                (n % 3) * (HD + 1):(n % 3) * (HD + 1) + HD + 1],
                    pext[0:bw, n, 0:mw],
                    vt[0:bw, n * (HD + 1):(n + 1) * (HD + 1)],
                    start=True, stop=True)
            rec = tsp.tile([128, NH], f32, tag="rec")
            out_sb = outp.tile([128, D], bf16, tag="osb")
            for b in range(3):
                nsl = 3 if b < 2 else 2
                pb = pvb[b]
                nc.vector.reciprocal(
                    rec[0:mw, 3 * b:3 * b + nsl],
                    bass.AP(pb.tensor, pb.offset + HD,
                            [[pb.ap[0][0], mw], [HD + 1, nsl]]))
                nc.vector.tensor_mul(
                    out_sb[0:mw, 3 * b * HD:(3 * b + nsl) * HD],
                    bass.AP(pb.tensor, pb.offset,
                            [[pb.ap[0][0], mw], [HD + 1, nsl], [1, HD]]),
                    bass.AP(rec.tensor, rec.offset + 3 * b,
                            [[rec.ap[0][0], mw], [1, nsl], [0, HD]]))
            # two 1KB-row strided stores (sub-2KB rows spray all 16 engines)
            for h in range(2):
                nc.gpsimd.dma_start(
                    bass.AP(out, c0 * ODP + h * (D // 2),
                            [[ODP, mw], [1, D // 2]]),
                    out_sb[0:mw, h * (D // 2):(h + 1) * (D // 2)])

        # ---- main loop over outer chunks ----
        xts_next = xts0
        for oi, (t0, ow) in enumerate(outers):
            kw = ow + HALO
            subs = _chunks_of(ow, MW)
            xts = xts_next

            # Q projections for all heads first (their weights arrive
            # first); rel logits r[q, F] interleave per head into one psum
            # bank, then get scattered (f32->bf16) into poisoned staging
            # rows and round tripped through two independent DRAM skew
            # planes (4 heads each): the skewed re-read returns
            # band[q, n, j] = rel value or -1e9, i.e. the rel term plus the
            # causal/window mask in one tile
            rel_ps = psS.tile([128, 512], f32, tag="st", name="rel_ps")
            QT, KT = [], []
            for n in range(NH):
                qt = qkp.tile([128, OUTER], bf16, tag=f"qt{n}")
                pq = psA.tile([128, 512], f32, tag="a", name="pq")
                for dc in range(8):
                    nc.tensor.matmul(pq[:, 0:ow],
                                     w_sb[("q", dc)][:, n * HD:(n + 1) * HD],
                                     xts[dc][:, HALO:HALO + ow],
                                     start=(dc == 0), stop=(dc == 7))
                nc.vector.tensor_copy(qt[:, 0:ow], pq[:, 0:ow])
                QT.append(qt)
                # rel logits for this head immediately: each skew plane's
                # scatter+write can then start as soon as its 4 heads of rel
                # are done (plane A after head 3, plane B after head 7)
                for si, (c0l, mw) in enumerate(subs):
                    nc.tensor.matmul(
                        rel_ps[0:mw, si * 104 + n * NF:si * 104 + (n + 1) * NF],
                        qt[:, c0l:c0l + mw], seT[:, n * NF:(n + 1) * NF],
                        start=True, stop=True)
                if n == 2 and pending:
                    # flush the previous outer's last PV tail once the PE
                    # queue has fresh projection work ahead of it
                    _emit_tail(pending.pop(0))

            # per subchunk: scatter both planes (DVE), write each plane and
            # read it back skewed via gpsimd SWDGE; the two planes live in
            # separate DRAM tensors so each read only waits its own write
            for si, (c0l, mw) in enumerate(subs):
                ci = (t0 // OUTER) * 4 + si
                slot = (ci % NSLOT) * SLOTSZ
                pb = pbands[si]
                for pl, rb in ((0, rbufA), (1, rbufB)):
                    nc.vector.tensor_copy(
                        bass.AP(pb.tensor, pb.offset + pl * RDP,
                                [[pb.ap[0][0], mw], [128, 4], [1, NF]]),
                        bass.AP(rel_ps.tensor,
                                rel_ps.offset + si * 104 + pl * 4 * NF,
                                [[rel_ps.ap[0][0], mw], [NF, 4], [1, NF]]))
                    nc.gpsimd.dma_start(
                        bass.AP(rb, slot, [[ROWP, mw], [1, RDP]]),
                        pb[0:mw, pl * RDP:(pl + 1) * RDP])
            bands = []
            for si, (c0l, mw) in enumerate(subs):
                ci = (t0 // OUTER) * 4 + si
                slot = (ci % NSLOT) * SLOTSZ
                bw = mw + HALO
                bandb = bandp.tile([128, NH, 128], bf16, tag="bandb")
                for pl, rb in ((0, rbufA), (1, rbufB)):
                    nc.gpsimd.dma_start(
                        bandb[0:mw, 4 * pl:4 * pl + 4, 0:bw],
                        bass.AP(rb, slot, [[RDP, mw], [128, 4], [1, bw]]))
                bands.append(bandb)
            # prefetch the next outer's x tiles (sync queue, behind the
            # band reads) so the next Q projections never wait on DMA
            if oi + 1 < len(outers):
                nt0, now_ = outers[oi + 1]
                nkw = now_ + HALO
                xts_next = []
                for dc in range(8):
                    xt = xpool.tile([128, OUTER + HALO], bf16, tag=f"xt{dc}")
                    nc.sync.dma_start(
                        xt[:, 0:nkw],
                        xT[dc * 128:(dc + 1) * 128, nt0:nt0 + nkw])
                    xts_next.append(xt)

            # K projections for all heads
            for n in range(NH):
                kt = qkp.tile([128, OUTER + HALO], bf16, tag=f"kt{n}")
                pk = psA.tile([128, 512], f32, tag="a", name="pk")
                for dc in range(8):
                    nc.tensor.matmul(pk[:, 0:kw],
                                     w_sb[("k", dc)][:, n * HD:(n + 1) * HD],
                                     xts[dc][:, 0:kw],
                                     start=(dc == 0), stop=(dc == 7))
                nc.scalar.copy(kt[:, 0:kw], pk[:, 0:kw])
                KT.append(kt)

            # ---- attention subchunks (software-pipelined: the PV tail of
            # chunk c is emitted after chunk c+1's head so the in-order PE
            # queue has work while ACT produces exp(c)) ----
            for si, (c0l, mw) in enumerate(subs):
                c0 = t0 + c0l
                bw = mw + HALO
                bandb = bands[si]

                # V first (PE work that needs no band data): the band
                # roundtrip gets the whole V phase as extra latency cover
                # before the first transpose consumes it
                vt = vp.tile([128, NH * (HD + 1)], bf16, tag="vt")
                vt3 = vt.rearrange("p (a b) -> p a b", a=NH)
                for hh in range(2):
                    pvv = psV.tile([128, 512], f32, tag="pv", name="pvv")
                    for dc in range(8):
                        nc.tensor.matmul(
                            pvv[0:bw, 0:512], xts[dc][:, c0l:c0l + bw],
                            w_sb[("v", dc)][:, hh * 512:(hh + 1) * 512],
                            start=(dc == 0), stop=(dc == 7))
                    nc.vector.tensor_copy(vt3[0:bw, hh * 4:(hh + 1) * 4, 0:HD],
                                          pvv[0:bw, 0:512])
                nc.gpsimd.memset(vt3[0:bw, :, HD:HD + 1], 1.0)

                # upcast the bf16 band to f32 per plane AFTER the vt copies:
                # it is consumed only by the transposes below, and keeping it
                # out of the DVE queue ahead of the vt copies stops PV tails
                # (whose PSUM banks rotate onto vt-copy consumers) from
                # transitively waiting on band-read DMA latency
                band = bandf.tile([128, NH, 128], f32, tag="band")
                for pl in range(2):
                    nc.vector.tensor_copy(band[0:mw, 4 * pl:4 * pl + 4, 0:bw],
                                          bandb[0:mw, 4 * pl:4 * pl + 4, 0:bw])

                if t0 == 0 and si == 0:
                    # global-start halo: keys j<12 are zero padding on
                    # first-half cores (halod = -1e9 there, 0 elsewhere)
                    nc.vector.tensor_scalar_add(
                        band[0:mw, :, 0:HALO], band[0:mw, :, 0:HALO],
                        halo_sb[0:mw, :])

                # S^T[j, q] = k_j . q_q per head, then the band tile (rel
                # term + mask, natural [q, j] orientation) is transposed by
                # the PE into the same accumulation group; start/stop pairs
                # stay adjacent per bank (start=True resets the whole bank's
                # accumulation group)
                st = [psS.tile([128, 512], f32, tag="st", name=f"st{i}")
                      for i in range(2)]
                for n in range(NH):
                    nc.tensor.matmul(
                        st[n // 4][0:bw, (n % 4) * MW:(n % 4) * MW + mw],
                        KT[n][:, c0l:c0l + bw], QT[n][:, c0l:c0l + mw],
                        start=True, stop=False)
                    nc.tensor.matmul(
                        st[n // 4][0:bw, (n % 4) * MW:(n % 4) * MW + mw],
                        band[0:mw, n, 0:bw], ident[0:mw, 0:mw],
                        is_transpose=True, start=False, stop=True)

                th = tsp.tile([128, NH, MW], f32, tag="th")
                for i in range(2):
                    nc.scalar.activation(
                        th[0:bw, i * 4:(i + 1) * 4, 0:mw],
                        bass.AP(st[i].tensor, st[i].offset,
                                [[st[i].ap[0][0], bw], [MW, 4], [1, mw]]),
                        AF.Tanh, scale=1.0 / CAP)
                pext = tsp.tile([128, NH, MW], bf16, tag="pext")
                nc.scalar.activation(pext[0:bw, :, 0:mw], th[0:bw, :, 0:mw],
                                     AF.Exp, scale=CAP)

                pending.append({"mw": mw, "bw": bw, "c0": c0,
                                "pext": pext, "vt": vt})
                if len(pending) > 1:
                    _emit_tail(pending.pop(0))
            # leave the final tail of this outer pending; it is flushed
            # early in the next outer's projection phase (or at the end)

        while pending:
            _emit_tail(pending.pop(0))

    _split_excess_waits(nc, mybir)
    return nc


def _split_excess_waits(nc, mybir, limit=1):
    """walrus rejects >2 sync waits on one instruction; hoist the excess
    onto same-engine NoOps inserted just before (engine queues are in-order,
    so waits on the NoOp happen-before the original instruction)."""
    nid = [0]
    for f in nc.m.functions:
        for blk in f.blocks:
            new = []
            for ins in blk.instructions:
                si = ins.sync_info
                if si is not None and si.on_wait and len(si.on_wait) > limit:
                    waits = list(si.on_wait)
                    keep = waits[-limit:]
                    rest = waits[:-limit]
                    while rest:
                        take, rest = rest[:limit], rest[limit:]
                        nop = mybir.InstNoOp(
                            name=f"waitnop-{nid[0]}", engine=ins.engine,
                            ins=[], outs=[])
                        nid[0] += 1
                        nop.sync_info = mybir.SyncInfo(
                            on_wait=take, on_update=[])
                        new.append(nop)
                    ins.sync_info = mybir.SyncInfo(
                        on_wait=keep, on_update=list(si.on_update))
                new.append(ins)
            blk.instructions[:] = new


_NC_CACHE = None


def _get_program():
    global _NC_CACHE
    if _NC_CACHE is None:
        _NC_CACHE = _build_program()
    return _NC_CACHE


def _host_inputs(x, mask, causal_valid_mask, wq, wk, wv, pos_proj):
    x = np.asarray(x, np.float32)
    wq = np.asarray(wq, np.float32) * QSC   # fold q scaling into wq
    wk = np.asarray(wk, np.float32)
    wv = np.asarray(wv, np.float32)
    pp = np.asarray(pos_proj, np.float32)   # [D, N, H]

    # constant sinusoidal table projected through pos_proj (input-independent)
    pos = np.arange(HALO, -1, -1, dtype=np.float32)  # [13]: 12..0
    nt = D // 2
    inc = np.log(10000.0) / (nt - 1)
    inv = np.exp(np.arange(nt, dtype=np.float32) * -inc)
    st = pos[:, None] * inv[None, :]
    sin_emb = np.concatenate([np.sin(st), np.cos(st)], axis=1)  # [13, D]
    # seT[h, n*13+F] = sum_d sin_emb[F, d] * pos_proj[d, n, h]
    se = np.einsum("fd,dnh->hnf", sin_emb, pp)       # [H, N, F]
    seTh = np.ascontiguousarray(se.reshape(HD, NH * NF)).astype(BF)

    wqb = wq.astype(BF)
    wkb = wk.astype(BF)
    wvb = wv.astype(BF)

    rpois = np.full(NSLOT * SLOTSZ, NEG, dtype=np.float32).astype(BF)
    in_maps = []
    for core in range(8):
        b, half = divmod(core, 2)
        t0 = half * TLOC
        lo = t0 - HALO
        if lo < 0:
            xs = np.concatenate(
                [np.zeros((HALO, D), np.float32), x[b, 0:t0 + TLOC]], axis=0)
        else:
            xs = x[b, lo:t0 + TLOC]
        halo = np.full((128, 1), NEG if half == 0 else 0.0, np.float32)
        in_maps.append({
            "xT": np.ascontiguousarray(xs.T).astype(BF),
            "wq": wqb, "wk": wkb, "wv": wvb,
            "seTd": seTh, "halod": halo,
            "identd": np.eye(128, dtype=np.float32),
            "rbufA": rpois, "rbufB": rpois,
        })
    return in_maps


_LAST_RESULTS = {"exec_time_ns": None}


def kernel(x, mask, causal_valid_mask, wq, wk, wv, pos_proj, _trace=False):
    from concourse import bass_utils
    nc = _get_program()
    in_maps = _host_inputs(x, mask, causal_valid_mask, wq, wk, wv, pos_proj)
    res = bass_utils.run_bass_kernel_spmd(
        nc, in_maps, core_ids=list(range(8)), trace=_trace)
    _LAST_RESULTS["exec_time_ns"] = res.exec_time_ns
    _LAST_RESULTS["profile_json"] = getattr(res, "profile_json", None)
    outs = [res.results[c]["out"] for c in range(8)]
    full = np.empty((B, T, NH, HD), np.float32)
    for core in range(8):
        b, half = divmod(core, 2)
        t0 = half * TLOC
        full[b, t0:t0 + TLOC] = np.asarray(
            outs[core][:, :D], dtype=np.float32).reshape(TLOC, NH, HD)
    return full


# revision 46
# speedup vs baseline: 1.0383x; 1.0016x over previous
"""Trainium2 Bass kernel for nn_AttentionBlock (blockwise local attention).

Per batch and head this is sliding-window causal attention with window 13
(query p attends to keys p-12..p), plus a relative-position logit term,
tanh soft-capping at 50, and key-validity masking.

Sharding: 8 cores = 4 batches x 2 T-halves. Each core computes all 8 heads
for 4080 queries of one batch half (12-row K/V halo) from host-pretransposed
x^T and the projection weights (q-scale folded into wq on host).

Per-core pipeline (bf16 matmuls, fp32 accumulation), chunked 116 queries at
a time so each chunk's 128-wide key band fits the PSUM partition dim:
  - Q^T,K^T = W^T @ x^T per head (PE); V in natural [key, head] layout (PE);
    rel logits r[q,F]=q_q.se_F (PE) interleaved per head into one psum bank
  - rel logits are scattered (DVE, f32->bf16) into persistent -1e9-poisoned
    SBUF row tiles and round-tripped through two independent DRAM skew
    planes (4 heads each, row pitch 514 written / 513 read back): the
    pitch-shifted re-read returns band[q,n,j] = rel value when
    0 <= q-j < 13 and -1e9 otherwise, i.e. the rel-position term plus the
    causal/window mask in one bf16 tile; separate DRAM tensors per plane
    keep each read gated only on its own plane's write
  - per subchunk: V matmuls first (band-latency cover), then per head
    S^T[j,q] = k_j.q_q into packed PSUM (4 heads/bank) with the upcast
    band tile transposed by the PE into the same accumulation group
    (start/stop pairs stay adjacent: start resets the bank's group)
  - tanh, exp (ACT, fused across heads) -> P^T in SBUF, bf16
  - P^T @ [V | 1] per head accumulates numerator and denominator (PE,
    deferred one subchunk so the in-order PE queue stays fed); reciprocal +
    broadcast multiply (DVE, bf16 out) -> two 1KB-row strided stores

Engine-queue discipline (the real bottlenecks were in-order queues and
DMA-engine assignment, not bandwidth): qt copies on DVE, kt copies on ACT,
x/weight loads on the SP HWDGE queue, and the whole band roundtrip plus
the output stores issued via gpsimd SWDGE -- SWDGE traffic spreads across
all 16 DMA engines (HWDGE pinned rbuf/out traffic to 4) and keeps DMA
issue off the ACT/SP queues. The bf16->f32 band upcast runs late (in the
attention loop) so in-order DVE progress never couples PV tails to
band-read DMA latency. PSUM pools are split (projections 2 / S+rel 2 /
V+PV 4 banks) so bank rotation never makes a projection wait on attention
consumers.
"""
import sys
import numpy as np

sys.path.insert(0, "/opt/trn_rl_repo")
import ml_dtypes  # noqa: E402

BF = ml_dtypes.bfloat16

B, T, D = 4, 8160, 1024
NH, HD = 8, 128
HALO = 12
CAP = 50.0
QSC = 1.0 / np.sqrt(HD)

TLOC = T // 2         # 4080 queries per core
KLOC = TLOC + HALO    # 4092
OUTER = 464           # queries per outer chunk (4 subchunks of 116)
MW = 116              # queries per attention subchunk (band = 116+12 = 128)
NF = 13               # relative-position offsets
ROWP = 4 * 128 + 2    # 514: skew-plane row pitch (4 heads per plane); the
                      # extra poison+gap cols keep the DRAM side strided so
                      # the DGE sprays all 16 engines
RDP = ROWP - 1        # 513: skewed read row pitch
ODP = D + 8           # 1032: padded out row pitch (strided store, 16 engines)
NSLOT = 8             # ring slots per plane (one per in-flight subchunk)
SLOTSZ = MW * ROWP    # bf16 elements per ring slot
NEG = -1.0e9


def _chunks_of(total, size):
    out = []
    o = 0
    while o < total:
        out.append((o, min(size, total - o)))
        o += size
    return out


def _build_program():
    import concourse.bass as bass
    import concourse.tile as tile
    from concourse import mybir
    from contextlib import ExitStack

    f32 = mybir.dt.float32
    bf16 = mybir.dt.bfloat16
    AF = mybir.ActivationFunctionType

    nc = bass.Bass(target_bir_lowering=False, debug=False)

    xT = nc.dram_tensor("xT", [D, KLOC], bf16, kind="ExternalInput")
    out = nc.dram_tensor("out", [TLOC, ODP], bf16, kind="ExternalOutput")
    rbufA = nc.dram_tensor("rbufA", [NSLOT * SLOTSZ], bf16,
                           kind="ExternalInput")
    rbufB = nc.dram_tensor("rbufB", [NSLOT * SLOTSZ], bf16,
                           kind="ExternalInput")
    wq = nc.dram_tensor("wq", [D, D], bf16, kind="ExternalInput")
    wk = nc.dram_tensor("wk", [D, D], bf16, kind="ExternalInput")
    wv = nc.dram_tensor("wv", [D, D], bf16, kind="ExternalInput")
    seTd = nc.dram_tensor("seTd", [HD, NH * NF], bf16, kind="ExternalInput")
    halod = nc.dram_tensor("halod", [128, 1], f32, kind="ExternalInput")
    identd = nc.dram_tensor("identd", [128, 128], f32, kind="ExternalInput")

    outers = _chunks_of(TLOC, OUTER)

    with tile.TileContext(nc) as tc, ExitStack() as ctx:
        const = ctx.enter_context(tc.tile_pool(name="const", bufs=1))
        wpool = ctx.enter_context(tc.tile_pool(name="wpool", bufs=1))
        xpool = ctx.enter_context(tc.tile_pool(name="xpool", bufs=2))
        qkp = ctx.enter_context(tc.tile_pool(name="qkp", bufs=2))
        vp = ctx.enter_context(tc.tile_pool(name="vp", bufs=2))
        bandp = ctx.enter_context(tc.tile_pool(name="bandp", bufs=4))
        bandf = ctx.enter_context(tc.tile_pool(name="bandf", bufs=4))
        tsp = ctx.enter_context(tc.tile_pool(name="tsp", bufs=2))
        outp = ctx.enter_context(tc.tile_pool(name="outp", bufs=2))
        psA = ctx.enter_context(tc.tile_pool(name="psA", bufs=2, space="PSUM"))
        psS = ctx.enter_context(tc.tile_pool(name="psS", bufs=2, space="PSUM"))
        psV = ctx.enter_context(tc.tile_pool(name="psV", bufs=4, space="PSUM"))

        # ---- constants / one-time init ----
        seT = const.tile([128, NH * NF], bf16, tag="seT")
        nc.sync.dma_start(seT[:], seTd[:, :])
        halo_sb = const.tile([128, 1], f32, tag="halo")
        nc.sync.dma_start(halo_sb[:], halod[:, :])
        ident = const.tile([128, 128], f32, tag="ident")
        nc.sync.dma_start(ident[:], identd[:, :])
        # four persistent skew-staging tiles (one per subchunk slot, two
        # 513-col planes each), poisoned once: the per-chunk rel scatter
        # only ever rewrites [0:13] of each 128-element segment, so the
        # poison in 13..127 (and col 512) survives; DRAM gap columns are
        # poisoned host-side (rbufA/rbufB arrive pre-filled with -1e9)
        pbands = [const.tile([128, 2 * RDP], bf16, tag=f"pband{i}",
                             name=f"pband{i}") for i in range(4)]
        for pb in pbands:
            nc.gpsimd.memset(pb[:, :], NEG)
        # Load order on the (FIFO) SP DMA queue is chosen so the PE can
        # start projecting as early as possible: wq interleaved with the
        # first outer's x tiles, then wk, wv.
        w_sb = {}
        xts0 = []
        kw0 = min(OUTER, TLOC) + HALO
        for dc in range(8):
            t = wpool.tile([128, D], bf16, tag=f"wq{dc}", name=f"wq{dc}")
            nc.sync.dma_start(t[:], wq[dc * 128:(dc + 1) * 128, :])
            w_sb[("q", dc)] = t
            xt = xpool.tile([128, OUTER + HALO], bf16, tag=f"xt{dc}")
            nc.sync.dma_start(xt[:, 0:kw0], xT[dc * 128:(dc + 1) * 128, 0:kw0])
            xts0.append(xt)
        for name, w in (("k", wk), ("v", wv)):
            for dc in range(8):
                t = wpool.tile([128, D], bf16, tag=f"w{name}{dc}",
                               name=f"w{name}{dc}")
                nc.sync.dma_start(t[:], w[dc * 128:(dc + 1) * 128, :])
                w_sb[(name, dc)] = t

        # deferred PV tails (see the attention loop below)
        pending = []

        def _emit_tail(p):
            mw, bw, c0 = p["mw"], p["bw"], p["c0"]
            pext, vt = p["pext"], p["vt"]
            pvb = [psV.tile([128, 512], f32, tag="pv", name=f"pvb{b}")
                   for b in range(3)]
            for n in range(NH):
                nc.tensor.matmul(
                    pvb[n // 3][0:mw,
                                (n % 3) * (HD + 1):(n % 3) * (HD + 1) + HD + 1],
                    pext[0:bw, n, 0:mw],
                    vt[0:bw, n * (HD + 1):(n + 1) * (HD + 1)],
                    start=True, stop=True)
            rec = tsp.tile([128, NH], f32, tag="rec")
            out_sb = outp.tile([128, D], bf16, tag="osb")
            for b in range(3):
                nsl = 3 if b < 2 else 2
                pb = pvb[b]
                nc.vector.reciprocal(
                    rec[0:mw, 3 * b:3 * b + nsl],
                    bass.AP(pb.tensor, pb.offset + HD,
                            [[pb.ap[0][0], mw], [HD + 1, nsl]]))
                nc.vector.tensor_mul(
                    out_sb[0:mw, 3 * b * HD:(3 * b + nsl) * HD],
                    bass.AP(pb.tensor, pb.offset,
                            [[pb.ap[0][0], mw], [HD + 1, nsl], [1, HD]]),
                    bass.AP(rec.tensor, rec.offset + 3 * b,
                            [[rec.ap[0][0], mw], [1, nsl], [0, HD]]))
            # two 1KB-row strided stores (sub-2KB rows spray all 16 engines);
            # the final outer's stores use the idle HWDGE queues to shorten
            # the end-of-kernel drain
            last = c0 >= TLOC - OUTER
            engs = (nc.sync, nc.scalar) if last else (nc.gpsimd, nc.gpsimd)
            for h in range(2):
                engs[h].dma_start(
                    bass.AP(out, c0 * ODP + h * (D // 2),
                            [[ODP, mw], [1, D // 2]]),
                    out_sb[0:mw, h * (D // 2):(h + 1) * (D // 2)])

        # ---- main loop over outer chunks ----
        xts_next = xts0
        for oi, (t0, ow) in enumerate(outers):
            kw = ow + HALO
            subs = _chunks_of(ow, MW)
            xts = xts_next

            # Q projections for all heads first (their weights arrive
            # first); rel logits r[q, F] interleave per head into one psum
            # bank, then get scattered (f32->bf16) into poisoned staging
            # rows and round tripped through two independent DRAM skew
            # planes (4 heads each): the skewed re-read returns
            # band[q, n, j] = rel value or -1e9, i.e. the rel term plus the
            # causal/window mask in one tile
            rel_ps = psS.tile([128, 512], f32, tag="st", name="rel_ps")
            QT, KT = [], []
            for n in range(NH):
                qt = qkp.tile([128, OUTER], bf16, tag=f"qt{n}")
                pq = psA.tile([128, 512], f32, tag="a", name="pq")
                for dc in range(8):
                    nc.tensor.matmul(pq[:, 0:ow],
                                     w_sb[("q", dc)][:, n * HD:(n + 1) * HD],
                                     xts[dc][:, HALO:HALO + ow],
                                     start=(dc == 0), stop=(dc == 7))
                nc.vector.tensor_copy(qt[:, 0:ow], pq[:, 0:ow])
                QT.append(qt)
                # rel logits for this head immediately: each skew plane's
                # scatter+write can then start as soon as its 4 heads of rel
                # are done (plane A after head 3, plane B after head 7)
                for si, (c0l, mw) in enumerate(subs):
                    nc.tensor.matmul(
                        rel_ps[0:mw, si * 104 + n * NF:si * 104 + (n + 1) * NF],
                        qt[:, c0l:c0l + mw], seT[:, n * NF:(n + 1) * NF],
                        start=True, stop=True)
                if n == 2 and pending:
                    # flush the previous outer's last PV tail once the PE
                    # queue has fresh projection work ahead of it
                    _emit_tail(pending.pop(0))

            # per subchunk: scatter both planes (DVE), write each plane and
            # read it back skewed via gpsimd SWDGE; the two planes live in
            # separate DRAM tensors so each read only waits its own write
            for si, (c0l, mw) in enumerate(subs):
                ci = (t0 // OUTER) * 4 + si
                slot = (ci % NSLOT) * SLOTSZ
                pb = pbands[si]
                for pl, rb in ((0, rbufA), (1, rbufB)):
                    nc.vector.tensor_copy(
                        bass.AP(pb.tensor, pb.offset + pl * RDP,
                                [[pb.ap[0][0], mw], [128, 4], [1, NF]]),
                        bass.AP(rel_ps.tensor,
                                rel_ps.offset + si * 104 + pl * 4 * NF,
                                [[rel_ps.ap[0][0], mw], [NF, 4], [1, NF]]))
                    nc.gpsimd.dma_start(
                        bass.AP(rb, slot, [[ROWP, mw], [1, RDP]]),
                        pb[0:mw, pl * RDP:(pl + 1) * RDP])
            bands = []
            for si, (c0l, mw) in enumerate(subs):
                ci = (t0 // OUTER) * 4 + si
                slot = (ci % NSLOT) * SLOTSZ
                bw = mw + HALO
                bandb = bandp.tile([128, NH, 128], bf16, tag="bandb")
                for pl, rb in ((0, rbufA), (1, rbufB)):
                    nc.gpsimd.dma_start(
                        bandb[0:mw, 4 * pl:4 * pl + 4, 0:bw],
                        bass.AP(rb, slot, [[RDP, mw], [128, 4], [1, bw]]))
                bands.append(bandb)
            # prefetch the next outer's x tiles (sync queue, behind the
            # band reads) so the next Q projections never wait on DMA
            if oi + 1 < len(outers):
                nt0, now_ = outers[oi + 1]
                nkw = now_ + HALO
                xts_next = []
                for dc in range(8):
                    xt = xpool.tile([128, OUTER + HALO], bf16, tag=f"xt{dc}")
                    nc.sync.dma_start(
                        xt[:, 0:nkw],
                        xT[dc * 128:(dc + 1) * 128, nt0:nt0 + nkw])
                    xts_next.append(xt)

            # K projections for all heads
            for n in range(NH):
                kt = qkp.tile([128, OUTER + HALO], bf16, tag=f"kt{n}")
                pk = psA.tile([128, 512], f32, tag="a", name="pk")
                for dc in range(8):
                    nc.tensor.matmul(pk[:, 0:kw],
                                     w_sb[("k", dc)][:, n * HD:(n + 1) * HD],
                                     xts[dc][:, 0:kw],
                                     start=(dc == 0), stop=(dc == 7))
                nc.scalar.copy(kt[:, 0:kw], pk[:, 0:kw])
                KT.append(kt)

            # ---- attention subchunks (software-pipelined: the PV tail of
            # chunk c is emitted after chunk c+1's head so the in-order PE
            # queue has work while ACT produces exp(c)) ----
            for si, (c0l, mw) in enumerate(subs):
                c0 = t0 + c0l
                bw = mw + HALO
                bandb = bands[si]

                # V first (PE work that needs no band data): the band
                # roundtrip gets the whole V phase as extra latency cover
                # before the first transpose consumes it
                vt = vp.tile([128, NH * (HD + 1)], bf16, tag="vt")
                vt3 = vt.rearrange("p (a b) -> p a b", a=NH)
                for hh in range(2):
                    pvv = psV.tile([128, 512], f32, tag="pv", name="pvv")
                    for dc in range(8):
                        nc.tensor.matmul(
                            pvv[0:bw, 0:512], xts[dc][:, c0l:c0l + bw],
                            w_sb[("v", dc)][:, hh * 512:(hh + 1) * 512],
                            start=(dc == 0), stop=(dc == 7))
                    nc.vector.tensor_copy(vt3[0:bw, hh * 4:(hh + 1) * 4, 0:HD],
                                          pvv[0:bw, 0:512])
                nc.gpsimd.memset(vt3[0:bw, :, HD:HD + 1], 1.0)

                # upcast the bf16 band to f32 per plane AFTER the vt copies:
                # it is consumed only by the transposes below, and keeping it
                # out of the DVE queue ahead of the vt copies stops PV tails
                # (whose PSUM banks rotate onto vt-copy consumers) from
                # transitively waiting on band-read DMA latency
                band = bandf.tile([128, NH, 128], f32, tag="band")
                for pl in range(2):
                    nc.vector.tensor_copy(band[0:mw, 4 * pl:4 * pl + 4, 0:bw],
                                          bandb[0:mw, 4 * pl:4 * pl + 4, 0:bw])

                if t0 == 0 and si == 0:
                    # global-start halo: keys j<12 are zero padding on
                    # first-half cores (halod = -1e9 there, 0 elsewhere)
                    nc.vector.tensor_scalar_add(
                        band[0:mw, :, 0:HALO], band[0:mw, :, 0:HALO],
                        halo_sb[0:mw, :])

                # S^T[j, q] = k_j . q_q per head, then the band tile (rel
                # term + mask, natural [q, j] orientation) is transposed by
                # the PE into the same accumulation group; start/stop pairs
                # stay adjacent per bank (start=True resets the whole bank's
                # accumulation group)
                st = [psS.tile([128, 512], f32, tag="st", name=f"st{i}")
                      for i in range(2)]
                for n in range(NH):
                    nc.tensor.matmul(
                        st[n // 4][0:bw, (n % 4) * MW:(n % 4) * MW + mw],
                        KT[n][:, c0l:c0l + bw], QT[n][:, c0l:c0l + mw],
                        start=True, stop=False)
                    nc.tensor.matmul(
                        st[n // 4][0:bw, (n % 4) * MW:(n % 4) * MW + mw],
                        band[0:mw, n, 0:bw], ident[0:mw, 0:mw],
                        is_transpose=True, start=False, stop=True)

                th = tsp.tile([128, NH, MW], f32, tag="th")
                for i in range(2):
                    nc.scalar.activation(
                        th[0:bw, i * 4:(i + 1) * 4, 0:mw],
                        bass.AP(st[i].tensor, st[i].offset,
                                [[st[i].ap[0][0], bw], [MW, 4], [1, mw]]),
                        AF.Tanh, scale=1.0 / CAP)
                pext = tsp.tile([128, NH, MW], bf16, tag="pext")
                nc.scalar.activation(pext[0:bw, :, 0:mw], th[0:bw, :, 0:mw],
                                     AF.Exp, scale=CAP)

                pending.append({"mw": mw, "bw": bw, "c0": c0,
                                "pext": pext, "vt": vt})
                if len(pending) > 1:
                    _emit_tail(pending.pop(0))
            # leave the final tail of this outer pending; it is flushed
            # early in the next outer's projection phase (or at the end)

        while pending:
            _emit_tail(pending.pop(0))

    _split_excess_waits(nc, mybir)
    return nc


def _split_excess_waits(nc, mybir, limit=1):
    """walrus rejects >2 sync waits on one instruction; hoist the excess
    onto same-engine NoOps inserted just before (engine queues are in-order,
    so waits on the NoOp happen-before the original instruction)."""
    nid = [0]
    for f in nc.m.functions:
        for blk in f.blocks:
            new = []
            for ins in blk.instructions:
                si = ins.sync_info
                if si is not None and si.on_wait and len(si.on_wait) > limit:
                    waits = list(si.on_wait)
                    keep = waits[-limit:]
                    rest = waits[:-limit]
                    while rest:
                        take, rest = rest[:limit], rest[limit:]
                        nop = mybir.InstNoOp(
                            name=f"waitnop-{nid[0]}", engine=ins.engine,
                            ins=[], outs=[])
                        nid[0] += 1
                        nop.sync_info = mybir.SyncInfo(
                            on_wait=take, on_update=[])
                        new.append(nop)
                    ins.sync_info = mybir.SyncInfo(
                        on_wait=keep, on_update=list(si.on_update))
                new.append(ins)
            blk.instructions[:] = new


_NC_CACHE = None


def _get_program():
    global _NC_CACHE
    if _NC_CACHE is None:
        _NC_CACHE = _build_program()
    return _NC_CACHE


def _host_inputs(x, mask, causal_valid_mask, wq, wk, wv, pos_proj):
    x = np.asarray(x, np.float32)
    wq = np.asarray(wq, np.float32) * QSC   # fold q scaling into wq
    wk = np.asarray(wk, np.float32)
    wv = np.asarray(wv, np.float32)
    pp = np.asarray(pos_proj, np.float32)   # [D, N, H]

    # constant sinusoidal table projected through pos_proj (input-independent)
    pos = np.arange(HALO, -1, -1, dtype=np.float32)  # [13]: 12..0
    nt = D // 2
    inc = np.log(10000.0) / (nt - 1)
    inv = np.exp(np.arange(nt, dtype=np.float32) * -inc)
    st = pos[:, None] * inv[None, :]
    sin_emb = np.concatenate([np.sin(st), np.cos(st)], axis=1)  # [13, D]
    # seT[h, n*13+F] = sum_d sin_emb[F, d] * pos_proj[d, n, h]
    se = np.einsum("fd,dnh->hnf", sin_emb, pp)       # [H, N, F]
    seTh = np.ascontiguousarray(se.reshape(HD, NH * NF)).astype(BF)

    wqb = wq.astype(BF)
    wkb = wk.astype(BF)
    wvb = wv.astype(BF)

    rpois = np.full(NSLOT * SLOTSZ, NEG, dtype=np.float32).astype(BF)
    in_maps = []
    for core in range(8):
        b, half = divmod(core, 2)
        t0 = half * TLOC
        lo = t0 - HALO
        if lo < 0:
            xs = np.concatenate(
                [np.zeros((HALO, D), np.float32), x[b, 0:t0 + TLOC]], axis=0)
        else:
            xs = x[b, lo:t0 + TLOC]
        halo = np.full((128, 1), NEG if half == 0 else 0.0, np.float32)
        in_maps.append({
            "xT": np.ascontiguousarray(xs.T).astype(BF),
            "wq": wqb, "wk": wkb, "wv": wvb,
            "seTd": seTh, "halod": halo,
            "identd": np.eye(128, dtype=np.float32),
            "rbufA": rpois, "rbufB": rpois,
        })
    return in_maps


_LAST_RESULTS = {"exec_time_ns": None}


def kernel(x, mask, causal_valid_mask, wq, wk, wv, pos_proj, _trace=False):
    from concourse import bass_utils
    nc = _get_program()
    in_maps = _host_inputs(x, mask, causal_valid_mask, wq, wk, wv, pos_proj)
    res = bass_utils.run_bass_kernel_spmd(
        nc, in_maps, core_ids=list(range(8)), trace=_trace)
    _LAST_RESULTS["exec_time_ns"] = res.exec_time_ns
    _LAST_RESULTS["profile_json"] = getattr(res, "profile_json", None)
    outs = [res.results[c]["out"] for c in range(8)]
    full = np.empty((B, T, NH, HD), np.float32)
    for core in range(8):
        b, half = divmod(core, 2)
        t0 = half * TLOC
        full[b, t0:t0 + TLOC] = np.asarray(
            outs[core][:, :D], dtype=np.float32).reshape(TLOC, NH, HD)
    return full
